# revision 9
# baseline (speedup 1.0000x reference)
"""Trainium2 Bass kernel for nn_Conv2dP4P4 (P4->P4 group-equivariant conv).

Math (verified vs reference):
  W2 = w.reshape(64,4,64,3,3).sum(1)                  # tap-sum absorbs the
                                                      # reference's group-sum
  out[b, 16q+m', i] = rot90( conv_valid(x[b,:,j], rot90(W2[16q:16q+16], k=i)),
                             k=-i )   with j = (q+i)%4

Device computes ONLY the conv (all 4 rotation blocks packed in M); the final
rot90(k=-i) of each 16-channel block is pure data movement, applied on the
host after the gather, so the device writes plain m-major conv-layout tiles
at full DMA burst size.

Per core (8 cores, batch-sharded: 2 batches x 4 group elements = 8 units):
  - slab S [128, H*W] bf16: partitions 0:64 = x[b,:,j], 64:128 = x shifted
    one row (+W).
  - 32 psum tiles [128, 504] f32 per unit, RT=4 output rows each (tile 31:
    RT=2 + 3 K=64 singles for the last row's kh=2 taps).  Each tile takes
    THREE K=128, M=128 matmul passes (one per kw):
      M cols   0:64  ("A"): taps (0,kw) via x[a] and (1,kw) via x[a+1]
               -> rows 4k..4k+3
      M cols 64:128  ("B"): tap (2,kw) via x[a+1]  -> rows 4k-1..4k+2
    75% PE utilization vs the 37.5% of a 6-pass M=64 scheme.
  - tiles 0..27: Act evicts the B half psum[64:128] -> bf16 ring[64:128]
    (aligned copy); DVE does ONE fused add per tile:
    E[rows 4k..4k+3] = psum_A[0:64, 0:504] + ring[B slots k.s1..s3,(k+1).s0]
    (the ring keeps consecutive tiles' B halves contiguous, so no separate
    cross-tile "tail" op).  TensorTensor allows mixed PSUM+SBUF inputs at
    different base partitions; both-PSUM or both-SBUF would not.
  - tiles 28..31: HOST-merged.  Act evicts the full [128, 504] psum to bf16
    and it is DMA'd raw; the host adds the A/B halves.  This drains the DVE
    queue before each unit boundary so the next unit's merges (which free
    PSUM banks for its tiles ~8+) are never backlogged.
  - E [64, 112*126] bf16 staging, flushed in three DMAs per unit (m-major,
    >=5KB contiguous runs -> full DMA bandwidth); host up-casts to f32.
"""
import sys
import numpy as np
import ml_dtypes

sys.path.insert(0, "/opt/trn_rl_repo")

B, C, G, H, W = 16, 64, 4, 128, 128
OUT = 64
HO = H - 2  # 126
NCORES = 8
BPC = B // NCORES  # batches per core
RT = 4
NT = 32            # 31 full tiles + 1 two-row tile
NDEV = 28          # tiles merged on device; 28..31 merged on host
HW = H * W
ER = NDEV * RT     # rows staged in E (112)

_cache = {}


def _build_weights(w: np.ndarray):
    """w: [256, 64, 3, 3] -> (WT [128, 12*128], WTL [64, 12*64]) bf16.

    WT column block (j*3+kw)*128:
      [0:64,   0:64] = LH[j, kw]      (kh0, applied to x[a])
      [64:128, 0:64] = LH[j, 3+kw]    (kh1, applied to x[a+1])
      [64:128,64:128]= LH[j, 6+kw]    (kh2 -> B half, rows a-1)
      [0:64,  64:128]= 0
    WTL column block (j*3+kw)*64 = LH[j, 6+kw] (last-row kh2 singles).
    """
    W2 = w.reshape(OUT, 4, C, 3, 3).sum(axis=1)  # [64, 64, 3, 3] f32
    iorder = [0, 2, 1, 3]
    LH = np.zeros((4, 9, C, 64), dtype=np.float32)  # [j, t, c, mhat]
    for j in range(4):
        for pos, i in enumerate(iorder):
            q = (j - i) % 4
            Ki = np.rot90(W2[16 * q:16 * (q + 1)], k=i, axes=(-2, -1))
            for kh in range(3):
                for kw in range(3):
                    t = kh * 3 + kw
                    LH[j, t, :, pos * 16:(pos + 1) * 16] = Ki[:, :, kh, kw].T
    WT = np.zeros((128, 12 * 128), dtype=np.float32)
    WTL = np.zeros((64, 12 * 64), dtype=np.float32)
    for j in range(4):
        for kw in range(3):
            col = (j * 3 + kw) * 128
            WT[0:64, col:col + 64] = LH[j, kw]
            WT[64:128, col:col + 64] = LH[j, 3 + kw]
            WT[64:128, col + 64:col + 128] = LH[j, 6 + kw]
            WTL[:, (j * 3 + kw) * 64:(j * 3 + kw) * 64 + 64] = LH[j, 6 + kw]
    return (WT.astype(ml_dtypes.bfloat16), WTL.astype(ml_dtypes.bfloat16))


def _build_program():
    import concourse.bass as bass
    import concourse.tile as tile
    from concourse import bacc, mybir

    nc = bacc.Bacc("TRN2", target_bir_lowering=False, debug=False)
    x_in = nc.dram_tensor("x_in", [BPC, C, G, H, W], mybir.dt.bfloat16,
                          kind="ExternalInput").ap()
    wt_in = nc.dram_tensor("wt_in", [128, 12 * 128], mybir.dt.bfloat16,
                           kind="ExternalInput").ap()
    wtl_in = nc.dram_tensor("wtl_in", [64, 12 * 64], mybir.dt.bfloat16,
                            kind="ExternalInput").ap()
    o_out = nc.dram_tensor("o_out", [BPC, G, OUT, ER, HO], mybir.dt.bfloat16,
                           kind="ExternalOutput").ap()
    c_out = nc.dram_tensor("c_out", [BPC, G, 4, 128, RT * 126],
                           mybir.dt.bfloat16, kind="ExternalOutput").ap()

    f32 = mybir.dt.float32
    bf16 = mybir.dt.bfloat16

    with tile.TileContext(nc, trace_sim=False) as tc:
        with tc.tile_pool(name="wtp", bufs=1) as wtp, \
             tc.tile_pool(name="slab", bufs=2) as slabp, \
             tc.tile_pool(name="ep", bufs=2) as ep, \
             tc.tile_pool(name="ring", bufs=1) as ringp, \
             tc.tile_pool(name="cp", bufs=3) as cp, \
             tc.tile_pool(name="psc", bufs=8, space="PSUM") as pscp:

            WT = wtp.tile([128, 12 * 128], bf16)
            # weight DMAs ride the Pool/SWDGE queue: off the HWDGE critical
            # path at startup.  Unit 0 only needs the j=0 columns -- load
            # them first so tile-0 Ldweights unblocks early.
            nc.gpsimd.dma_start(WT[:, 0:384], wt_in[:, 0:384])
            nc.gpsimd.dma_start(WT[:, 384:], wt_in[:, 384:])
            WTL = wtp.tile([64, 12 * 64], bf16)
            nc.gpsimd.dma_start(WTL[:], wtl_in)
            # B-half ring for device-merged tiles, partitions 64:128
            ring = ringp.tile([128, NDEV * 504], bf16)

            def slab_src(u):
                b, j = u // 4, u % 4
                return x_in[b, :, j].rearrange("c h w -> c (h w)")

            def load_slab_piece(S, src, piece):
                if piece == 0:
                    nc.sync.dma_start(S[0:64, 0:HW // 2], src[:, 0:HW // 2])
                elif piece == 1:
                    nc.sync.dma_start(S[0:64, HW // 2:HW], src[:, HW // 2:])
                elif piece == 2:
                    nc.sync.dma_start(S[64:128, 0:HW // 2],
                                      src[:, W:W + HW // 2])
                else:
                    nc.sync.dma_start(S[64:128, HW // 2:HW - W],
                                      src[:, W + HW // 2:])

            NU = BPC * 4
            S_cur = slabp.tile([128, HW + 8], bf16, tag="slab")
            # unit 0: fine-grained load so the first matmuls start early
            src0 = slab_src(0)
            for qtr in range(4):
                a, bnd = 4096 * qtr, 4096 * (qtr + 1)
                nc.sync.dma_start(S_cur[0:64, a:bnd], src0[:, a:bnd])
                hi = min(bnd, HW - W)
                nc.sync.dma_start(S_cur[64:128, a:hi], src0[:, W + a:W + hi])

            for u in range(NU):
                b, j = u // 4, u % 4
                S = S_cur
                if u + 1 < NU:
                    S_next = slabp.tile([128, HW + 8], bf16, tag="slab",
                                        name="S_next")
                else:
                    S_next = None

                E = ep.tile([64, ER * HO], bf16, tag="E")
                prev_pt = None

                for k in range(NT):
                    if S_next is not None and k in (6, 12, 18, 24):
                        load_slab_piece(S_next, slab_src(u + 1), k // 6 - 1)

                    pt = pscp.tile([128, RT * 126], f32, tag="conv")
                    if k < NT - 1:
                        p4 = pt[:].rearrange("m (s x) -> m s x", s=RT)
                        for kw in range(3):
                            base = RT * k * W + kw
                            nc.tensor.matmul(
                                p4,
                                WT[:, (j * 3 + kw) * 128:
                                   (j * 3 + kw) * 128 + 128],
                                S[:, base:base + RT * 128]
                                .rearrange("c (s x) -> c s x", s=RT)
                                [:, :, 0:126],
                                start=(kw == 0), stop=(kw == 2),
                                skip_group_check=True)
                    else:
                        # rows 124, 125: two windows + kh2 singles for row 125
                        p2 = pt[:, 0:252].rearrange("m (s x) -> m s x", s=2)
                        for kw in range(3):
                            base = RT * k * W + kw
                            nc.tensor.matmul(
                                p2,
                                WT[:, (j * 3 + kw) * 128:
                                   (j * 3 + kw) * 128 + 128],
                                S[:, base:base + 2 * 128]
                                .rearrange("c (s x) -> c s x", s=2)
                                [:, :, 0:126],
                                start=(kw == 0), stop=False,
                                skip_group_check=True)
                        for kw in range(3):
                            base = (H - 1) * W + kw
                            nc.tensor.matmul(
                                pt[0:64, 126:252],
                                WTL[:, (j * 3 + kw) * 64:
                                    (j * 3 + kw) * 64 + 64],
                                S[0:64, base:base + 126],
                                start=False, stop=(kw == 2),
                                skip_group_check=True)

                    if k < NDEV:
                        # Act: evict B half into the ring (aligned)
                        nc.scalar.copy(ring[64:128, 504 * k:504 * k + 504],
                                       pt[64:128, 0:504])
                        # DVE: fused merge of tile k-1 (k<=27 -> reads ring
                        # slots k-1 and k)
                        if 1 <= k:
                            nc.vector.tensor_add(
                                E[:, 504 * (k - 1):504 * k],
                                prev_pt[0:64, 0:504],
                                ring[64:128, 504 * (k - 1) + 126:
                                     504 * (k - 1) + 630])
                    else:
                        # host-merged tile: full-tile eviction + raw dump
                        wk = 504 if k < NT - 1 else 252
                        Ct = cp.tile([128, RT * 126], bf16, tag="C")
                        nc.scalar.copy(Ct[:, 0:wk], pt[:, 0:wk])
                        if k == NDEV:
                            # merge(27): main (same-tile B slots) ...
                            nc.vector.tensor_add(
                                E[:, 504 * 27:504 * 27 + 378],
                                prev_pt[0:64, 0:378],
                                ring[64:128, 504 * 27 + 126:504 * 27 + 504])
                            # ... + tail row 111 (B s0 from evicted tile 28)
                            nc.vector.tensor_add(
                                E[:, 504 * 27 + 378:504 * 28],
                                prev_pt[0:64, 378:504],
                                Ct[64:128, 0:126])
                            nc.sync.dma_start(
                                o_out[b, j, :, 84:112, :]
                                .rearrange("m u v -> m (u v)"),
                                E[:, 10584:14112])
                        nc.sync.dma_start(
                            c_out[b, j, k - NDEV][:, 0:wk], Ct[:, 0:wk])
                    prev_pt = pt

                    if k == 11:
                        nc.sync.dma_start(
                            o_out[b, j, :, 0:42, :]
                            .rearrange("m u v -> m (u v)"),
                            E[:, 0:5292])
                    elif k == 21:
                        nc.sync.dma_start(
                            o_out[b, j, :, 42:84, :]
                            .rearrange("m u v -> m (u v)"),
                            E[:, 5292:10584])

                S_cur = S_next

    nc.compile()
    return nc


def kernel(x: np.ndarray, w: np.ndarray) -> np.ndarray:
    from concourse.bass_utils import run_bass_kernel_spmd

    if "nc" not in _cache:
        _cache["nc"] = _build_program()
    nc = _cache["nc"]

    wt, wtl = _build_weights(np.asarray(w, dtype=np.float32))
    xb = np.ascontiguousarray(
        np.asarray(x, dtype=np.float32).astype(ml_dtypes.bfloat16))
    in_maps = [{"x_in": xb[c * BPC:(c + 1) * BPC], "wt_in": wt,
                "wtl_in": wtl}
               for c in range(NCORES)]
    _cache["in_maps"] = in_maps
    res = run_bass_kernel_spmd(nc, in_maps, list(range(NCORES)))

    iorder = [0, 2, 1, 3]
    out = np.empty((B, OUT, G, HO, HO), dtype=np.float32)
    conv = np.empty((OUT, HO, HO), dtype=np.float32)
    for c in range(NCORES):
        oc = np.asarray(res.results[c]["o_out"]).astype(np.float32)
        cc = np.asarray(res.results[c]["c_out"]).astype(np.float32)
        for bi in range(BPC):
            for j in range(4):
                conv[:, 0:ER] = oc[bi, j]
                # host merge of tiles 28..31: A[t][:, s] is row 4(28+t)+s,
                # B[t][:, s'] is row 4(28+t)+s'-1 (kh2 part)
                Ah = cc[bi, j, :, 0:64, :].reshape(4, 64, RT, 126)
                Bh = cc[bi, j, :, 64:128, :].reshape(4, 64, RT, 126)
                for r in range(ER, HO):
                    t, s = r // 4 - NDEV, r % 4
                    row = Ah[t][:, s]
                    if r < HO - 1:
                        kp, sp = (r + 1) // 4 - NDEV, (r + 1) % 4
                        row = row + Bh[kp][:, sp]
                    conv[:, r] = row
                for pos, i in enumerate(iorder):
                    q = (j - i) % 4
                    out[c * BPC + bi, 16 * q:16 * (q + 1), i] = np.rot90(
                        conv[16 * pos:16 * (pos + 1)], k=-i, axes=(-2, -1))
    return out


# revision 15
# speedup vs baseline: 1.0416x; 1.0416x over previous
"""Trainium2 Bass kernel for nn_Conv2dP4P4 (P4->P4 group-equivariant conv).

Math (verified vs reference):
  W2 = w.reshape(64,4,64,3,3).sum(1)                  # tap-sum absorbs the
                                                      # reference's group-sum
  out[b, 16q+m', i] = rot90( conv_valid(x[b,:,j], rot90(W2[16q:16q+16], k=i)),
                             k=-i )   with j = (q+i)%4

Device computes ONLY the conv (all 4 rotation blocks packed in M); the final
rot90(k=-i) of each 16-channel block is pure data movement, applied on the
host after the gather, so the device writes plain m-major conv-layout tiles
at full DMA burst size.

Per core (8 cores, batch-sharded: 2 batches x 4 group elements = 8 units):
  - slab S [128, H*W] bf16: partitions 0:64 = x[b,:,j], 64:128 = x shifted
    one row (+W).
  - 32 psum tiles [128, 504] f32 per unit, RT=4 output rows each (tile 31:
    RT=2 + 3 K=64 singles for the last row's kh=2 taps).  Each tile takes
    THREE K=128, M=128 matmul passes (one per kw):
      M cols   0:64  ("A"): taps (0,kw) via x[a] and (1,kw) via x[a+1]
               -> rows 4k..4k+3
      M cols 64:128  ("B"): tap (2,kw) via x[a+1]  -> rows 4k-1..4k+2
    75% PE utilization vs the 37.5% of a 6-pass M=64 scheme.
  - tiles 4..31: Act evicts the B half psum[64:128] -> bf16 ring[64:128]
    (aligned copy); DVE does ONE fused add per tile:
    E[rows 4k..4k+3] = psum_A[0:64, 0:504] + ring[B slots k.s1..s3,(k+1).s0]
    (the ring keeps consecutive tiles' B halves contiguous, so no separate
    cross-tile "tail" op).  TensorTensor allows mixed PSUM+SBUF inputs at
    different base partitions; both-PSUM or both-SBUF would not.
  - tiles 0..3: HOST-merged.  Act evicts the full [128, 504] psum to bf16,
    DMA'd raw; the host adds the A/B halves.  Putting the host tiles at the
    UNIT START means their PSUM banks free via Act alone, so the next
    unit's tile 8..11 allocations never wait on a DVE merge backlog, and
    DVE gets a 4-tile grace period per unit to drain.
  - E [64, 111*126] bf16 staging (rows 15..125), flushed in three DMAs per
    unit (m-major, >=4.5KB contiguous runs); host up-casts to f32.
"""
import sys
import numpy as np
import ml_dtypes

sys.path.insert(0, "/opt/trn_rl_repo")

B, C, G, H, W = 16, 64, 4, 128, 128
OUT = 64
HO = H - 2  # 126
NCORES = 8
BPC = B // NCORES  # batches per core
RT = 4
NT = 32            # 31 full tiles + 1 two-row tile
NHOST = 4          # tiles 0..3 and 28..31 merged on host; 4..27 device
HW = H * W
ER = HO - NHOST * RT + 1   # rows 15..125 staged in E (111)

_cache = {}


def _build_weights(w: np.ndarray):
    """w: [256, 64, 3, 3] -> (WT [128, 12*128], WTL [64, 12*64]) bf16.

    WT column block (j*3+kw)*128:
      [0:64,   0:64] = LH[j, kw]      (kh0, applied to x[a])
      [64:128, 0:64] = LH[j, 3+kw]    (kh1, applied to x[a+1])
      [64:128,64:128]= LH[j, 6+kw]    (kh2 -> B half, rows a-1)
      [0:64,  64:128]= 0
    WTL column block (j*3+kw)*64 = LH[j, 6+kw] (last-row kh2 singles).
    """
    W2 = w.reshape(OUT, 4, C, 3, 3).sum(axis=1)  # [64, 64, 3, 3] f32
    iorder = [0, 2, 1, 3]
    LH = np.zeros((4, 9, C, 64), dtype=np.float32)  # [j, t, c, mhat]
    for j in range(4):
        for pos, i in enumerate(iorder):
            q = (j - i) % 4
            Ki = np.rot90(W2[16 * q:16 * (q + 1)], k=i, axes=(-2, -1))
            for kh in range(3):
                for kw in range(3):
                    t = kh * 3 + kw
                    LH[j, t, :, pos * 16:(pos + 1) * 16] = Ki[:, :, kh, kw].T
    WT = np.zeros((128, 12 * 128), dtype=np.float32)
    WTL = np.zeros((64, 12 * 64), dtype=np.float32)
    for j in range(4):
        for kw in range(3):
            col = (j * 3 + kw) * 128
            WT[0:64, col:col + 64] = LH[j, kw]
            WT[64:128, col:col + 64] = LH[j, 3 + kw]
            WT[64:128, col + 64:col + 128] = LH[j, 6 + kw]
            WTL[:, (j * 3 + kw) * 64:(j * 3 + kw) * 64 + 64] = LH[j, 6 + kw]
    return (WT.astype(ml_dtypes.bfloat16), WTL.astype(ml_dtypes.bfloat16))


def _build_program():
    import concourse.bass as bass
    import concourse.tile as tile
    from concourse import bacc, mybir

    nc = bacc.Bacc("TRN2", target_bir_lowering=False, debug=False)
    x_in = nc.dram_tensor("x_in", [BPC, C, G, H, W], mybir.dt.bfloat16,
                          kind="ExternalInput").ap()
    wt_in = nc.dram_tensor("wt_in", [128, 12 * 128], mybir.dt.bfloat16,
                           kind="ExternalInput").ap()
    wtl_in = nc.dram_tensor("wtl_in", [64, 12 * 64], mybir.dt.bfloat16,
                            kind="ExternalInput").ap()
    o_out = nc.dram_tensor("o_out", [BPC, G, OUT, 97, HO], mybir.dt.bfloat16,
                           kind="ExternalOutput").ap()
    c_out = nc.dram_tensor("c_out", [BPC, G, 8, 128, RT * 126],
                           mybir.dt.bfloat16, kind="ExternalOutput").ap()
    r15_out = nc.dram_tensor("r15_out", [BPC, G, 64, HO], mybir.dt.bfloat16,
                             kind="ExternalOutput").ap()

    f32 = mybir.dt.float32
    bf16 = mybir.dt.bfloat16

    with tile.TileContext(nc, trace_sim=False) as tc:
        with tc.tile_pool(name="wtp", bufs=1) as wtp, \
             tc.tile_pool(name="slab", bufs=2) as slabp, \
             tc.tile_pool(name="ep", bufs=2) as ep, \
             tc.tile_pool(name="ring", bufs=1) as ringp, \
             tc.tile_pool(name="cp", bufs=6) as cp, \
             tc.tile_pool(name="psc", bufs=8, space="PSUM") as pscp:

            WT = wtp.tile([128, 12 * 128], bf16)
            # weight DMAs ride the Pool/SWDGE queue: off the HWDGE critical
            # path at startup.  Unit 0 only needs the j=0 columns -- load
            # them first so tile-0 Ldweights unblocks early.
            nc.gpsimd.dma_start(WT[:, 0:384], wt_in[:, 0:384])
            nc.gpsimd.dma_start(WT[:, 384:], wt_in[:, 384:])
            WTL = wtp.tile([64, 12 * 64], bf16)
            nc.gpsimd.dma_start(WTL[:], wtl_in)
            # B-half ring for device-merged tiles, partitions 64:128
            ring = ringp.tile([128, NT * 504], bf16)

            def slab_src(u):
                b, j = u // 4, u % 4
                return x_in[b, :, j].rearrange("c h w -> c (h w)")

            # slab as TWO tiles (cols 0:8200 / 8192:16384, 8-col overlap):
            # tiles 0..15 depend only on the lo tile's writers, so a late
            # hi-half DMA can never stall the next unit's first matmuls
            LOW = 8200
            HIB = HW // 2  # 8192

            def load_slab_piece(Slo, Shi, src, piece):
                if piece == 0:
                    nc.sync.dma_start(Slo[0:64, 0:LOW], src[:, 0:LOW])
                elif piece == 1:
                    nc.sync.dma_start(Slo[64:128, 0:LOW],
                                      src[:, W:W + LOW])
                elif piece == 2:
                    nc.sync.dma_start(Shi[0:64, 0:HW - HIB], src[:, HIB:])
                else:
                    nc.sync.dma_start(Shi[64:128, 0:HW - HIB - W],
                                      src[:, HIB + W:])

            NU = BPC * 4
            S_cur = (slabp.tile([128, LOW + 8], bf16, tag="slo",
                                name="S_lo0"),
                     slabp.tile([128, HW - HIB + 8], bf16, tag="shi",
                                name="S_hi0"))
            # unit 0: a tiny head tile (cols 0:1032) for tiles 0..1 lets the
            # first matmuls start ~3us in, while the full lo/hi load streams
            src0 = slab_src(0)
            S_head = wtp.tile([128, 1040], bf16, name="S_head")
            nc.sync.dma_start(S_head[0:64, 0:1032], src0[:, 0:1032])
            nc.sync.dma_start(S_head[64:128, 0:1032], src0[:, W:W + 1032])
            for piece in range(4):
                load_slab_piece(S_cur[0], S_cur[1], src0, piece)

            for u in range(NU):
                b, j = u // 4, u % 4
                S_lo, S_hi = S_cur
                if u + 1 < NU:
                    S_next = (slabp.tile([128, LOW + 8], bf16, tag="slo",
                                         name="S_lo_next"),
                              slabp.tile([128, HW - HIB + 8], bf16,
                                         tag="shi", name="S_hi_next"))
                else:
                    S_next = None

                E = ep.tile([64, ER * HO], bf16, tag="E")
                prev_pt = None
                cts = []

                def eoff(r):
                    return (r - 15) * 126

                for k in range(NT):
                    if S_next is not None and k in (0, 2, 5, 8):
                        load_slab_piece(S_next[0], S_next[1],
                                        slab_src(u + 1),
                                        {0: 0, 2: 1, 5: 2, 8: 3}[k])

                    if u == 0 and k < 2:
                        S, off = S_head, 0
                    elif k < 16:
                        S, off = S_lo, 0
                    else:
                        S, off = S_hi, HIB
                    pt = pscp.tile([128, RT * 126], f32, tag="conv")
                    if k < NT - 1:
                        p4 = pt[:].rearrange("m (s x) -> m s x", s=RT)
                        for kw in range(3):
                            base = RT * k * W + kw - off
                            nc.tensor.matmul(
                                p4,
                                WT[:, (j * 3 + kw) * 128:
                                   (j * 3 + kw) * 128 + 128],
                                S[:, base:base + RT * 128]
                                .rearrange("c (s x) -> c s x", s=RT)
                                [:, :, 0:126],
                                start=(kw == 0), stop=(kw == 2),
                                skip_group_check=True)
                    else:
                        # rows 124, 125: two windows + kh2 singles for row 125
                        p2 = pt[:, 0:252].rearrange("m (s x) -> m s x", s=2)
                        for kw in range(3):
                            base = RT * k * W + kw - off
                            nc.tensor.matmul(
                                p2,
                                WT[:, (j * 3 + kw) * 128:
                                   (j * 3 + kw) * 128 + 128],
                                S[:, base:base + 2 * 128]
                                .rearrange("c (s x) -> c s x", s=2)
                                [:, :, 0:126],
                                start=(kw == 0), stop=False,
                                skip_group_check=True)
                        for kw in range(3):
                            base = (H - 1) * W + kw - off
                            nc.tensor.matmul(
                                pt[0:64, 126:252],
                                WTL[:, (j * 3 + kw) * 64:
                                    (j * 3 + kw) * 64 + 64],
                                S[0:64, base:base + 126],
                                start=False, stop=(kw == 2),
                                skip_group_check=True)

                    if k < NHOST:
                        # host-merged head tile: full eviction (dumped at
                        # k=9, after all slab-piece DMAs are on the queue)
                        Ct = cp.tile([128, RT * 126], bf16, tag="C")
                        nc.scalar.copy(Ct[:], pt[:])
                        cts.append(Ct)
                    elif k < 28:
                        # Act: evict B half into the ring (aligned)
                        nc.scalar.copy(ring[64:128, 504 * k:504 * k + 504],
                                       pt[64:128, 0:504])
                        if k > NHOST:
                            # DVE: fused merge of tile k-1 (rows 4(k-1)..+3)
                            nc.vector.tensor_add(
                                E[:, eoff(4 * (k - 1)):eoff(4 * k)],
                                prev_pt[0:64, 0:504],
                                ring[64:128, 504 * (k - 1) + 126:
                                     504 * (k - 1) + 630])
                    else:
                        # host-merged tail tile
                        wk = 504 if k < NT - 1 else 252
                        Ct = cp.tile([128, RT * 126], bf16, tag="C")
                        nc.scalar.copy(Ct[:, 0:wk], pt[:, 0:wk])
                        if k == 28:
                            # merge(27): rows 108..110 + row 111 tail (B s0
                            # read from the evicted tile-28 SBUF copy)
                            nc.vector.tensor_add(
                                E[:, eoff(108):eoff(111)],
                                prev_pt[0:64, 0:378],
                                ring[64:128, 504 * 27 + 126:504 * 27 + 504])
                            nc.vector.tensor_add(
                                E[:, eoff(111):eoff(112)],
                                prev_pt[0:64, 378:504],
                                Ct[64:128, 0:126])
                            # rows 100..111 complete
                            nc.sync.dma_start(
                                o_out[b, j, :, 85:97, :]
                                .rearrange("m u v -> m (u v)"),
                                E[:, 10710:12222])
                        nc.sync.dma_start(
                            c_out[b, j, 4 + k - 28][:, 0:wk], Ct[:, 0:wk])
                    prev_pt = pt

                    if k == 9:
                        for tt in range(NHOST):
                            nc.sync.dma_start(c_out[b, j, tt], cts[tt][:])
                        # row 15's kh2 part (B s0 of tile 4) for the host
                        nc.sync.dma_start(
                            r15_out[b, j],
                            ring[64:128, 504 * NHOST:504 * NHOST + 126])
                    elif k == 13:
                        # rows 16..51 complete after merge(12)
                        nc.sync.dma_start(
                            o_out[b, j, :, 1:37, :]
                            .rearrange("m u v -> m (u v)"),
                            E[:, 126:4662])
                    elif k == 25:
                        # rows 52..99 complete after merge(24)
                        nc.sync.dma_start(
                            o_out[b, j, :, 37:85, :]
                            .rearrange("m u v -> m (u v)"),
                            E[:, 4662:10710])

                S_cur = S_next

    nc.compile()
    return nc


def kernel(x: np.ndarray, w: np.ndarray) -> np.ndarray:
    from concourse.bass_utils import run_bass_kernel_spmd

    if "nc" not in _cache:
        _cache["nc"] = _build_program()
    nc = _cache["nc"]

    wt, wtl = _build_weights(np.asarray(w, dtype=np.float32))
    xb = np.ascontiguousarray(
        np.asarray(x, dtype=np.float32).astype(ml_dtypes.bfloat16))
    in_maps = [{"x_in": xb[c * BPC:(c + 1) * BPC], "wt_in": wt,
                "wtl_in": wtl}
               for c in range(NCORES)]
    _cache["in_maps"] = in_maps
    res = run_bass_kernel_spmd(nc, in_maps, list(range(NCORES)))

    iorder = [0, 2, 1, 3]
    out = np.empty((B, OUT, G, HO, HO), dtype=np.float32)
    conv = np.empty((OUT, HO, HO), dtype=np.float32)
    for c in range(NCORES):
        oc = np.asarray(res.results[c]["o_out"]).astype(np.float32)
        cc = np.asarray(res.results[c]["c_out"]).astype(np.float32)
        rr = np.asarray(res.results[c]["r15_out"]).astype(np.float32)
        for bi in range(BPC):
            for j in range(4):
                conv[:, 16:112] = oc[bi, j][:, 1:]
                # host merge: A[t][:, s] is row 4K+s, B[t][:, s'] is row
                # 4K+s'-1 (kh2 part); t 0..3 = tiles 0..3, 4..7 = 28..31
                Ah = cc[bi, j, :, 0:64, :].reshape(8, 64, RT, 126)
                Bh = cc[bi, j, :, 64:128, :].reshape(8, 64, RT, 126)
                for r in range(NHOST * RT - 1):
                    kp, sp = (r + 1) // 4, (r + 1) % 4
                    conv[:, r] = Ah[r // 4][:, r % 4] + Bh[kp][:, sp]
                conv[:, 15] = Ah[3][:, 3] + rr[bi, j]
                for r in range(112, 125):
                    t, s = r // 4 - 28, r % 4
                    kp, sp = (r + 1) // 4 - 28, (r + 1) % 4
                    conv[:, r] = Ah[4 + t][:, s] + Bh[4 + kp][:, sp]
                conv[:, 125] = Ah[7][:, 1]
                for pos, i in enumerate(iorder):
                    q = (j - i) % 4
                    out[c * BPC + bi, 16 * q:16 * (q + 1), i] = np.rot90(
                        conv[16 * pos:16 * (pos + 1)], k=-i, axes=(-2, -1))
    return out


# revision 16
# speedup vs baseline: 1.1256x; 1.0807x over previous
"""Trainium2 Bass kernel for nn_Conv2dP4P4 (P4->P4 group-equivariant conv).

Math (verified vs reference):
  W2 = w.reshape(64,4,64,3,3).sum(1)                  # tap-sum absorbs the
                                                      # reference's group-sum
  out[b, 16q+m', i] = rot90( conv_valid(x[b,:,j], rot90(W2[16q:16q+16], k=i)),
                             k=-i )   with j = (q+i)%4

Device computes ONLY the conv (all 4 rotation blocks packed in M); the final
rot90(k=-i) of each 16-channel block is pure data movement, applied on the
host after the gather, so the device writes plain m-major conv-layout tiles
at full DMA burst size.

Per core (8 cores, batch-sharded: 2 batches x 4 group elements = 8 units):
  - slab S [128, H*W] bf16: partitions 0:64 = x[b,:,j], 64:128 = x shifted
    one row (+W).
  - 32 psum tiles [128, 504] f32 per unit, RT=4 output rows each (tile 31:
    RT=2 + 3 K=64 singles for the last row's kh=2 taps).  Each tile takes
    THREE K=128, M=128 matmul passes (one per kw):
      M cols   0:64  ("A"): taps (0,kw) via x[a] and (1,kw) via x[a+1]
               -> rows 4k..4k+3
      M cols 64:128  ("B"): tap (2,kw) via x[a+1]  -> rows 4k-1..4k+2
    75% PE utilization vs the 37.5% of a 6-pass M=64 scheme.
  - tiles 4..31: Act evicts the B half psum[64:128] -> bf16 ring[64:128]
    (aligned copy); DVE does ONE fused add per tile:
    E[rows 4k..4k+3] = psum_A[0:64, 0:504] + ring[B slots k.s1..s3,(k+1).s0]
    (the ring keeps consecutive tiles' B halves contiguous, so no separate
    cross-tile "tail" op).  TensorTensor allows mixed PSUM+SBUF inputs at
    different base partitions; both-PSUM or both-SBUF would not.
  - tiles 0..3: HOST-merged.  Act evicts the full [128, 504] psum to bf16,
    DMA'd raw; the host adds the A/B halves.  Putting the host tiles at the
    UNIT START means their PSUM banks free via Act alone, so the next
    unit's tile 8..11 allocations never wait on a DVE merge backlog, and
    DVE gets a 4-tile grace period per unit to drain.
  - E [64, 111*126] bf16 staging (rows 15..125), flushed in three DMAs per
    unit (m-major, >=4.5KB contiguous runs); host up-casts to f32.
"""
import sys
import numpy as np
import ml_dtypes

sys.path.insert(0, "/opt/trn_rl_repo")

B, C, G, H, W = 16, 64, 4, 128, 128
OUT = 64
HO = H - 2  # 126
NCORES = 8
BPC = B // NCORES  # batches per core
RT = 4
NT = 32            # 31 full tiles + 1 two-row tile
NHOST = 4          # tiles 0..3 and 28..31 merged on host; 4..27 device
HW = H * W
ER = HO - NHOST * RT + 1   # rows 15..125 staged in E (111)

_cache = {}


def _build_weights(w: np.ndarray):
    """w: [256, 64, 3, 3] -> (WT [128, 12*128], WTL [64, 12*64]) bf16.

    WT column block (j*3+kw)*128:
      [0:64,   0:64] = LH[j, kw]      (kh0, applied to x[a])
      [64:128, 0:64] = LH[j, 3+kw]    (kh1, applied to x[a+1])
      [64:128,64:128]= LH[j, 6+kw]    (kh2 -> B half, rows a-1)
      [0:64,  64:128]= 0
    WTL column block (j*3+kw)*64 = LH[j, 6+kw] (last-row kh2 singles).
    """
    W2 = w.reshape(OUT, 4, C, 3, 3).sum(axis=1)  # [64, 64, 3, 3] f32
    iorder = [0, 2, 1, 3]
    LH = np.zeros((4, 9, C, 64), dtype=np.float32)  # [j, t, c, mhat]
    for j in range(4):
        for pos, i in enumerate(iorder):
            q = (j - i) % 4
            Ki = np.rot90(W2[16 * q:16 * (q + 1)], k=i, axes=(-2, -1))
            for kh in range(3):
                for kw in range(3):
                    t = kh * 3 + kw
                    LH[j, t, :, pos * 16:(pos + 1) * 16] = Ki[:, :, kh, kw].T
    WT = np.zeros((128, 12 * 128), dtype=np.float32)
    WTL = np.zeros((64, 12 * 64), dtype=np.float32)
    for j in range(4):
        for kw in range(3):
            col = (j * 3 + kw) * 128
            WT[0:64, col:col + 64] = LH[j, kw]
            WT[64:128, col:col + 64] = LH[j, 3 + kw]
            WT[64:128, col + 64:col + 128] = LH[j, 6 + kw]
            WTL[:, (j * 3 + kw) * 64:(j * 3 + kw) * 64 + 64] = LH[j, 6 + kw]
    return (WT.astype(ml_dtypes.bfloat16), WTL.astype(ml_dtypes.bfloat16))


def _build_program():
    import concourse.bass as bass
    import concourse.tile as tile
    from concourse import bacc, mybir

    nc = bacc.Bacc("TRN2", target_bir_lowering=False, debug=False)
    x_in = nc.dram_tensor("x_in", [BPC, C, G, H, W], mybir.dt.bfloat16,
                          kind="ExternalInput").ap()
    wt_in = nc.dram_tensor("wt_in", [128, 12 * 128], mybir.dt.bfloat16,
                           kind="ExternalInput").ap()
    wtl_in = nc.dram_tensor("wtl_in", [64, 12 * 64], mybir.dt.bfloat16,
                            kind="ExternalInput").ap()
    o_out = nc.dram_tensor("o_out", [BPC, G, OUT, 97, HO], mybir.dt.bfloat16,
                           kind="ExternalOutput").ap()
    c_out = nc.dram_tensor("c_out", [BPC, G, 2, 128, 4 * 504],
                           mybir.dt.bfloat16, kind="ExternalOutput").ap()
    r15_out = nc.dram_tensor("r15_out", [BPC, G, 64, HO], mybir.dt.bfloat16,
                             kind="ExternalOutput").ap()

    f32 = mybir.dt.float32
    bf16 = mybir.dt.bfloat16

    with tile.TileContext(nc, trace_sim=False) as tc:
        with tc.tile_pool(name="wtp", bufs=1) as wtp, \
             tc.tile_pool(name="slab", bufs=2) as slabp, \
             tc.tile_pool(name="ep", bufs=2) as ep, \
             tc.tile_pool(name="ring", bufs=1) as ringp, \
             tc.tile_pool(name="cp", bufs=6) as cp, \
             tc.tile_pool(name="psc", bufs=8, space="PSUM") as pscp:

            WT = wtp.tile([128, 12 * 128], bf16)
            # weight DMAs ride the Pool/SWDGE queue: off the HWDGE critical
            # path at startup.  Unit 0 only needs the j=0 columns -- load
            # them first so tile-0 Ldweights unblocks early.
            nc.gpsimd.dma_start(WT[:, 0:384], wt_in[:, 0:384])
            nc.gpsimd.dma_start(WT[:, 384:], wt_in[:, 384:])
            WTL = wtp.tile([64, 12 * 64], bf16)
            nc.gpsimd.dma_start(WTL[:], wtl_in)
            # B-half ring for device-merged tiles, partitions 64:128
            ring = ringp.tile([128, NT * 504], bf16)

            def slab_src(u):
                b, j = u // 4, u % 4
                return x_in[b, :, j].rearrange("c h w -> c (h w)")

            # slab as TWO tiles (cols 0:8200 / 8192:16384, 8-col overlap):
            # tiles 0..15 depend only on the lo tile's writers, so a late
            # hi-half DMA can never stall the next unit's first matmuls
            LOW = 8200
            HIB = HW // 2  # 8192

            def load_slab_piece(Slo, Shi, src, piece):
                if piece == 0:
                    nc.sync.dma_start(Slo[0:64, 0:LOW], src[:, 0:LOW])
                elif piece == 1:
                    nc.sync.dma_start(Slo[64:128, 0:LOW],
                                      src[:, W:W + LOW])
                elif piece == 2:
                    nc.sync.dma_start(Shi[0:64, 0:HW - HIB], src[:, HIB:])
                else:
                    nc.sync.dma_start(Shi[64:128, 0:HW - HIB - W],
                                      src[:, HIB + W:])

            NU = BPC * 4
            S_cur = (slabp.tile([128, LOW + 8], bf16, tag="slo",
                                name="S_lo0"),
                     slabp.tile([128, HW - HIB + 8], bf16, tag="shi",
                                name="S_hi0"))
            # unit 0: a tiny head tile (cols 0:1032) for tiles 0..1 lets the
            # first matmuls start ~3us in, while the full lo/hi load streams
            src0 = slab_src(0)
            S_head = wtp.tile([128, 2064], bf16, name="S_head")
            nc.sync.dma_start(S_head[0:64, 0:2056], src0[:, 0:2056])
            nc.sync.dma_start(S_head[64:128, 0:2056], src0[:, W:W + 2056])
            for h in range(2):
                for qtr in range(2):
                    a, bnd = (0, 4100) if qtr == 0 else (4100, LOW)
                    po = 0 if h == 0 else W
                    nc.sync.dma_start(S_cur[0][64 * h:64 * (h + 1), a:bnd],
                                      src0[:, po + a:po + bnd])
            load_slab_piece(S_cur[0], S_cur[1], src0, 2)
            load_slab_piece(S_cur[0], S_cur[1], src0, 3)

            for u in range(NU):
                b, j = u // 4, u % 4
                S_lo, S_hi = S_cur
                if u + 1 < NU:
                    S_next = (slabp.tile([128, LOW + 8], bf16, tag="slo",
                                         name="S_lo_next"),
                              slabp.tile([128, HW - HIB + 8], bf16,
                                         tag="shi", name="S_hi_next"))
                else:
                    S_next = None

                E = ep.tile([64, ER * HO], bf16, tag="E")
                prev_pt = None
                CtH = cp.tile([128, 4 * 504], bf16, tag="C", name="CtH")
                CtT = None

                def eoff(r):
                    return (r - 15) * 126

                for k in range(NT):
                    if S_next is not None and k in (0, 2, 5, 8):
                        load_slab_piece(S_next[0], S_next[1],
                                        slab_src(u + 1),
                                        {0: 0, 2: 1, 5: 2, 8: 3}[k])

                    if u == 0 and k < 4:
                        S, off = S_head, 0
                    elif k < 16:
                        S, off = S_lo, 0
                    else:
                        S, off = S_hi, HIB
                    pt = pscp.tile([128, RT * 126], f32, tag="conv")
                    if k < NT - 1:
                        p4 = pt[:].rearrange("m (s x) -> m s x", s=RT)
                        for kw in range(3):
                            base = RT * k * W + kw - off
                            nc.tensor.matmul(
                                p4,
                                WT[:, (j * 3 + kw) * 128:
                                   (j * 3 + kw) * 128 + 128],
                                S[:, base:base + RT * 128]
                                .rearrange("c (s x) -> c s x", s=RT)
                                [:, :, 0:126],
                                start=(kw == 0), stop=(kw == 2),
                                skip_group_check=True)
                    else:
                        # rows 124, 125: two windows + kh2 singles for row 125
                        p2 = pt[:, 0:252].rearrange("m (s x) -> m s x", s=2)
                        for kw in range(3):
                            base = RT * k * W + kw - off
                            nc.tensor.matmul(
                                p2,
                                WT[:, (j * 3 + kw) * 128:
                                   (j * 3 + kw) * 128 + 128],
                                S[:, base:base + 2 * 128]
                                .rearrange("c (s x) -> c s x", s=2)
                                [:, :, 0:126],
                                start=(kw == 0), stop=False,
                                skip_group_check=True)
                        for kw in range(3):
                            base = (H - 1) * W + kw - off
                            nc.tensor.matmul(
                                pt[0:64, 126:252],
                                WTL[:, (j * 3 + kw) * 64:
                                    (j * 3 + kw) * 64 + 64],
                                S[0:64, base:base + 126],
                                start=False, stop=(kw == 2),
                                skip_group_check=True)

                    if k < NHOST:
                        # host-merged head tile: full eviction (single dump
                        # at k=9, after all slab-piece DMAs are queued).
                        # tiles 0,1 evict on DVE (idle at unit start),
                        # 2,3 on Act.
                        dst = CtH[:, 504 * k:504 * (k + 1)]
                        if k < 2:
                            nc.vector.tensor_copy(dst, pt[:])
                        else:
                            nc.scalar.copy(dst, pt[:])
                    elif k < 28:
                        # Act: evict B half into the ring (aligned)
                        nc.scalar.copy(ring[64:128, 504 * k:504 * k + 504],
                                       pt[64:128, 0:504])
                        if k > NHOST:
                            # DVE: fused merge of tile k-1 (rows 4(k-1)..+3)
                            nc.vector.tensor_add(
                                E[:, eoff(4 * (k - 1)):eoff(4 * k)],
                                prev_pt[0:64, 0:504],
                                ring[64:128, 504 * (k - 1) + 126:
                                     504 * (k - 1) + 630])
                    else:
                        # host-merged tail tile: 28,29 evict on DVE (idle
                        # at unit end), 30,31 on Act; single dump at k=31
                        wk = 504 if k < NT - 1 else 252
                        if CtT is None:
                            CtT = cp.tile([128, 4 * 504], bf16, tag="C",
                                          name="CtT")
                        dst = CtT[:, 504 * (k - 28):504 * (k - 28) + wk]
                        if k < 30:
                            nc.vector.tensor_copy(dst, pt[:, 0:wk])
                        else:
                            nc.scalar.copy(dst, pt[:, 0:wk])
                        if k == 28:
                            # merge(27): rows 108..110 + row 111 tail (B s0
                            # read from the evicted tile-28 SBUF copy)
                            nc.vector.tensor_add(
                                E[:, eoff(108):eoff(111)],
                                prev_pt[0:64, 0:378],
                                ring[64:128, 504 * 27 + 126:504 * 27 + 504])
                            nc.vector.tensor_add(
                                E[:, eoff(111):eoff(112)],
                                prev_pt[0:64, 378:504],
                                CtT[64:128, 0:126])
                            # rows 100..111 complete
                            nc.sync.dma_start(
                                o_out[b, j, :, 85:97, :]
                                .rearrange("m u v -> m (u v)"),
                                E[:, 10710:12222])
                        elif k == NT - 1:
                            nc.sync.dma_start(c_out[b, j, 1], CtT[:])
                    prev_pt = pt

                    if k == 9:
                        nc.sync.dma_start(c_out[b, j, 0], CtH[:])
                        # row 15's kh2 part (B s0 of tile 4) for the host
                        nc.sync.dma_start(
                            r15_out[b, j],
                            ring[64:128, 504 * NHOST:504 * NHOST + 126])
                    elif k == 13:
                        # rows 16..51 complete after merge(12)
                        nc.sync.dma_start(
                            o_out[b, j, :, 1:37, :]
                            .rearrange("m u v -> m (u v)"),
                            E[:, 126:4662])
                    elif k == 25:
                        # rows 52..99 complete after merge(24)
                        nc.sync.dma_start(
                            o_out[b, j, :, 37:85, :]
                            .rearrange("m u v -> m (u v)"),
                            E[:, 4662:10710])

                S_cur = S_next

    nc.compile()
    return nc


def kernel(x: np.ndarray, w: np.ndarray) -> np.ndarray:
    from concourse.bass_utils import run_bass_kernel_spmd

    if "nc" not in _cache:
        _cache["nc"] = _build_program()
    nc = _cache["nc"]

    wt, wtl = _build_weights(np.asarray(w, dtype=np.float32))
    xb = np.ascontiguousarray(
        np.asarray(x, dtype=np.float32).astype(ml_dtypes.bfloat16))
    in_maps = [{"x_in": xb[c * BPC:(c + 1) * BPC], "wt_in": wt,
                "wtl_in": wtl}
               for c in range(NCORES)]
    _cache["in_maps"] = in_maps
    res = run_bass_kernel_spmd(nc, in_maps, list(range(NCORES)))

    iorder = [0, 2, 1, 3]
    out = np.empty((B, OUT, G, HO, HO), dtype=np.float32)
    conv = np.empty((OUT, HO, HO), dtype=np.float32)
    for c in range(NCORES):
        oc = np.asarray(res.results[c]["o_out"]).astype(np.float32)
        cc = np.asarray(res.results[c]["c_out"]).astype(np.float32)
        rr = np.asarray(res.results[c]["r15_out"]).astype(np.float32)
        for bi in range(BPC):
            for j in range(4):
                conv[:, 16:112] = oc[bi, j][:, 1:]
                # host merge: A[t][:, s] is row 4K+s, B[t][:, s'] is row
                # 4K+s'-1 (kh2 part); t 0..3 = tiles 0..3, 4..7 = 28..31
                Ah = cc[bi, j, :, 0:64, :].reshape(2, 64, 4, RT, 126)
                Bh = cc[bi, j, :, 64:128, :].reshape(2, 64, 4, RT, 126)
                for r in range(NHOST * RT - 1):
                    kp, sp = (r + 1) // 4, (r + 1) % 4
                    conv[:, r] = Ah[0][:, r // 4, r % 4] + Bh[0][:, kp, sp]
                conv[:, 15] = Ah[0][:, 3, 3] + rr[bi, j]
                for r in range(112, 125):
                    t, s = r // 4 - 28, r % 4
                    kp, sp = (r + 1) // 4 - 28, (r + 1) % 4
                    conv[:, r] = Ah[1][:, t, s] + Bh[1][:, kp, sp]
                conv[:, 125] = Ah[1][:, 3, 1]
                for pos, i in enumerate(iorder):
                    q = (j - i) % 4
                    out[c * BPC + bi, 16 * q:16 * (q + 1), i] = np.rot90(
                        conv[16 * pos:16 * (pos + 1)], k=-i, axes=(-2, -1))
    return out


# revision 22
# speedup vs baseline: 1.1313x; 1.0050x over previous
"""Trainium2 Bass kernel for nn_Conv2dP4P4 (P4->P4 group-equivariant conv).

Math (verified vs reference):
  W2 = w.reshape(64,4,64,3,3).sum(1)                  # tap-sum absorbs the
                                                      # reference's group-sum
  out[b, 16q+m', i] = rot90( conv_valid(x[b,:,j], rot90(W2[16q:16q+16], k=i)),
                             k=-i )   with j = (q+i)%4

Device computes ONLY the conv (all 4 rotation blocks packed in M); the final
rot90(k=-i) of each 16-channel block is pure data movement, applied on the
host after the gather, so the device writes plain m-major conv-layout tiles
at full DMA burst size.

Per core (8 cores, batch-sharded: 2 batches x 4 group elements = 8 units):
  - slab S [128, H*W] bf16: partitions 0:64 = x[b,:,j], 64:128 = x shifted
    one row (+W).
  - 32 psum tiles [128, 504] f32 per unit, RT=4 output rows each (tile 31:
    RT=2 + 3 K=64 singles for the last row's kh=2 taps).  Each tile takes
    THREE K=128, M=128 matmul passes (one per kw):
      M cols   0:64  ("A"): taps (0,kw) via x[a] and (1,kw) via x[a+1]
               -> rows 4k..4k+3
      M cols 64:128  ("B"): tap (2,kw) via x[a+1]  -> rows 4k-1..4k+2
    75% PE utilization vs the 37.5% of a 6-pass M=64 scheme.
  - tiles 4..31: Act evicts the B half psum[64:128] -> bf16 ring[64:128]
    (aligned copy); DVE does ONE fused add per tile:
    E[rows 4k..4k+3] = psum_A[0:64, 0:504] + ring[B slots k.s1..s3,(k+1).s0]
    (the ring keeps consecutive tiles' B halves contiguous, so no separate
    cross-tile "tail" op).  TensorTensor allows mixed PSUM+SBUF inputs at
    different base partitions; both-PSUM or both-SBUF would not.
  - tiles 0..3: HOST-merged.  Act evicts the full [128, 504] psum to bf16,
    DMA'd raw; the host adds the A/B halves.  Putting the host tiles at the
    UNIT START means their PSUM banks free via Act alone, so the next
    unit's tile 8..11 allocations never wait on a DVE merge backlog, and
    DVE gets a 4-tile grace period per unit to drain.
  - E [64, 111*126] bf16 staging (rows 15..125), flushed in three DMAs per
    unit (m-major, >=4.5KB contiguous runs); host up-casts to f32.
"""
import sys
import numpy as np
import ml_dtypes

sys.path.insert(0, "/opt/trn_rl_repo")

B, C, G, H, W = 16, 64, 4, 128, 128
OUT = 64
HO = H - 2  # 126
NCORES = 8
BPC = B // NCORES  # batches per core
RT = 4
NT = 32            # 31 full tiles + 1 two-row tile
NHOST = 4          # tiles 0..3 and 28..31 merged on host; 4..27 device
HW = H * W
ER = HO - NHOST * RT + 1   # rows 15..125 staged in E (111)

_cache = {}


def _build_weights(w: np.ndarray):
    """w: [256, 64, 3, 3] -> (WT [128, 12*128], WTL [64, 12*64]) bf16.

    WT column block (j*3+kw)*128:
      [0:64,   0:64] = LH[j, kw]      (kh0, applied to x[a])
      [64:128, 0:64] = LH[j, 3+kw]    (kh1, applied to x[a+1])
      [64:128,64:128]= LH[j, 6+kw]    (kh2 -> B half, rows a-1)
      [0:64,  64:128]= 0
    WTL column block (j*3+kw)*64 = LH[j, 6+kw] (last-row kh2 singles).
    """
    W2 = w.reshape(OUT, 4, C, 3, 3).sum(axis=1)  # [64, 64, 3, 3] f32
    iorder = [0, 2, 1, 3]
    LH = np.zeros((4, 9, C, 64), dtype=np.float32)  # [j, t, c, mhat]
    for j in range(4):
        for pos, i in enumerate(iorder):
            q = (j - i) % 4
            Ki = np.rot90(W2[16 * q:16 * (q + 1)], k=i, axes=(-2, -1))
            for kh in range(3):
                for kw in range(3):
                    t = kh * 3 + kw
                    LH[j, t, :, pos * 16:(pos + 1) * 16] = Ki[:, :, kh, kw].T
    WT = np.zeros((128, 12 * 128), dtype=np.float32)
    WTL = np.zeros((64, 12 * 64), dtype=np.float32)
    for j in range(4):
        for kw in range(3):
            col = (j * 3 + kw) * 128
            WT[0:64, col:col + 64] = LH[j, kw]
            WT[64:128, col:col + 64] = LH[j, 3 + kw]
            WT[64:128, col + 64:col + 128] = LH[j, 6 + kw]
            WTL[:, (j * 3 + kw) * 64:(j * 3 + kw) * 64 + 64] = LH[j, 6 + kw]
    return (WT.astype(ml_dtypes.bfloat16), WTL.astype(ml_dtypes.bfloat16))


def _build_program():
    import concourse.bass as bass
    import concourse.tile as tile
    from concourse import bacc, mybir

    nc = bacc.Bacc("TRN2", target_bir_lowering=False, debug=False)
    x_in = nc.dram_tensor("x_in", [BPC, C, G, H, W], mybir.dt.bfloat16,
                          kind="ExternalInput").ap()
    wt_in = nc.dram_tensor("wt_in", [128, 12 * 128], mybir.dt.bfloat16,
                           kind="ExternalInput").ap()
    wtl_in = nc.dram_tensor("wtl_in", [64, 12 * 64], mybir.dt.bfloat16,
                            kind="ExternalInput").ap()
    o_out = nc.dram_tensor("o_out", [BPC, G, OUT, 97, HO], mybir.dt.bfloat16,
                           kind="ExternalOutput").ap()
    c_out = nc.dram_tensor("c_out", [BPC, G, 2, 128, 4 * 504],
                           mybir.dt.bfloat16, kind="ExternalOutput").ap()
    r15_out = nc.dram_tensor("r15_out", [BPC, G, 64, HO], mybir.dt.bfloat16,
                             kind="ExternalOutput").ap()

    f32 = mybir.dt.float32
    bf16 = mybir.dt.bfloat16

    with tile.TileContext(nc, trace_sim=False) as tc:
        with tc.tile_pool(name="wtp", bufs=1) as wtp, \
             tc.tile_pool(name="slab", bufs=2) as slabp, \
             tc.tile_pool(name="ep", bufs=2) as ep, \
             tc.tile_pool(name="ring", bufs=1) as ringp, \
             tc.tile_pool(name="cp", bufs=6) as cp, \
             tc.tile_pool(name="psc", bufs=8, space="PSUM") as pscp:

            WT = wtp.tile([128, 12 * 128], bf16)
            # weight DMAs ride the Pool/SWDGE queue: off the HWDGE critical
            # path at startup.  Unit 0 only needs the j=0 columns -- load
            # them first so tile-0 Ldweights unblocks early.
            nc.gpsimd.dma_start(WT[:, 0:384], wt_in[:, 0:384])
            nc.gpsimd.dma_start(WT[:, 384:], wt_in[:, 384:])
            WTL = wtp.tile([64, 12 * 64], bf16)
            nc.gpsimd.dma_start(WTL[:], wtl_in)
            # B-half ring for device-merged tiles, partitions 64:128
            ring = ringp.tile([128, NT * 504], bf16)

            def slab_src(u):
                b, j = u // 4, u % 4
                return x_in[b, :, j].rearrange("c h w -> c (h w)")

            # slab as TWO tiles (cols 0:8200 / 8192:16384, 8-col overlap):
            # tiles 0..15 depend only on the lo tile's writers, so a late
            # hi-half DMA can never stall the next unit's first matmuls
            LOW = 8200
            HIB = HW // 2  # 8192

            def load_slab_piece(Slo, Shi, src, piece):
                if piece == 0:
                    nc.sync.dma_start(Slo[0:64, 0:LOW], src[:, 0:LOW])
                elif piece == 1:
                    nc.sync.dma_start(Slo[64:128, 0:LOW],
                                      src[:, W:W + LOW])
                elif piece == 2:
                    nc.sync.dma_start(Shi[0:64, 0:HW - HIB], src[:, HIB:])
                else:
                    nc.sync.dma_start(Shi[64:128, 0:HW - HIB - W],
                                      src[:, HIB + W:])

            NU = BPC * 4
            S_cur = (slabp.tile([128, LOW + 8], bf16, tag="slo",
                                name="S_lo0"),
                     slabp.tile([128, HW - HIB + 8], bf16, tag="shi",
                                name="S_hi0"))
            # unit 0: a tiny head tile (cols 0:1032) for tiles 0..1 lets the
            # first matmuls start ~3us in, while the full lo/hi load streams
            src0 = slab_src(0)
            # small head tile covers tiles 0..3; unit 0 then never reads
            # S_lo cols < 2048, so skip loading them
            S_head = wtp.tile([128, 2064], bf16, name="S_head")
            nc.sync.dma_start(S_head[0:64, 0:2056], src0[:, 0:2056])
            nc.sync.dma_start(S_head[64:128, 0:2056], src0[:, W:W + 2056])
            for qtr in range(2):
                a, bnd = (2048, 5124) if qtr == 0 else (5124, LOW)
                for h in range(2):
                    po = 0 if h == 0 else W
                    nc.sync.dma_start(S_cur[0][64 * h:64 * (h + 1), a:bnd],
                                      src0[:, po + a:po + bnd])
            load_slab_piece(S_cur[0], S_cur[1], src0, 2)
            load_slab_piece(S_cur[0], S_cur[1], src0, 3)


            for u in range(NU):
                b, j = u // 4, u % 4
                S_lo, S_hi = S_cur
                if u + 1 < NU:
                    S_next = (slabp.tile([128, LOW + 8], bf16, tag="slo",
                                         name="S_lo_next"),
                              slabp.tile([128, HW - HIB + 8], bf16,
                                         tag="shi", name="S_hi_next"))
                else:
                    S_next = None

                E = ep.tile([64, ER * HO], bf16, tag="E")
                prev_pt = None
                CtH = cp.tile([128, 4 * 504], bf16, tag="C", name="CtH")
                CtT = None

                def eoff(r):
                    return (r - 15) * 126

                def finish27(pt27):
                    # row 111 first: its B part (CtT s0) was evicted tiles
                    # ago, so it needn't wait for evict(27); then rows
                    # 108..110 via the ring, then flush rows 108..111
                    nc.vector.tensor_add(
                        E[:, eoff(111):eoff(112)], pt27[0:64, 378:504],
                        CtT[64:128, 0:126])
                    nc.vector.tensor_add(
                        E[:, eoff(108):eoff(111)], pt27[0:64, 0:378],
                        ring[64:128, 504 * 27 + 126:504 * 27 + 504])
                    nc.sync.dma_start(
                        o_out[b, j, :, 93:97, :]
                        .rearrange("m u v -> m (u v)"),
                        E[:, eoff(108):eoff(112)])

                korder = list(range(NT))
                if u == NU - 1:
                    korder = [0, 1, 2, 3, 28, 29, 30, 31] + list(range(4, 28))
                for k in korder:
                    if S_next is not None and k in (0, 2, 5, 8):
                        load_slab_piece(S_next[0], S_next[1],
                                        slab_src(u + 1),
                                        {0: 0, 2: 1, 5: 2, 8: 3}[k])

                    if u == 0 and k < 4:
                        S, off = S_head, 0
                    elif k < 16:
                        S, off = S_lo, 0
                    else:
                        S, off = S_hi, HIB
                    pt = pscp.tile([128, RT * 126], f32, tag="conv")
                    if k < NT - 1:
                        p4 = pt[:].rearrange("m (s x) -> m s x", s=RT)
                        for kw in range(3):
                            base = RT * k * W + kw - off
                            nc.tensor.matmul(
                                p4,
                                WT[:, (j * 3 + kw) * 128:
                                   (j * 3 + kw) * 128 + 128],
                                S[:, base:base + RT * 128]
                                .rearrange("c (s x) -> c s x", s=RT)
                                [:, :, 0:126],
                                start=(kw == 0), stop=(kw == 2),
                                skip_group_check=True)
                    else:
                        # rows 124, 125: two windows + kh2 singles for row 125
                        p2 = pt[:, 0:252].rearrange("m (s x) -> m s x", s=2)
                        for kw in range(3):
                            base = RT * k * W + kw - off
                            nc.tensor.matmul(
                                p2,
                                WT[:, (j * 3 + kw) * 128:
                                   (j * 3 + kw) * 128 + 128],
                                S[:, base:base + 2 * 128]
                                .rearrange("c (s x) -> c s x", s=2)
                                [:, :, 0:126],
                                start=(kw == 0), stop=False,
                                skip_group_check=True)
                        for kw in range(3):
                            base = (H - 1) * W + kw - off
                            nc.tensor.matmul(
                                pt[0:64, 126:252],
                                WTL[:, (j * 3 + kw) * 64:
                                    (j * 3 + kw) * 64 + 64],
                                S[0:64, base:base + 126],
                                start=False, stop=(kw == 2),
                                skip_group_check=True)

                    if k < NHOST:
                        # host-merged head tile: full eviction (single dump
                        # at k=9, after all slab-piece DMAs are queued).
                        # tiles 0,1 evict on DVE (idle at unit start),
                        # 2,3 on Act.
                        dst = CtH[:, 504 * k:504 * (k + 1)]
                        if k < 2:
                            nc.vector.tensor_copy(dst, pt[:])
                        else:
                            nc.scalar.copy(dst, pt[:])
                    elif k < 28:
                        # Act: evict B half into the ring (aligned)
                        nc.scalar.copy(ring[64:128, 504 * k:504 * k + 504],
                                       pt[64:128, 0:504])
                        if k > NHOST:
                            # DVE: fused merge of tile k-1 (rows 4(k-1)..+3)
                            nc.vector.tensor_add(
                                E[:, eoff(4 * (k - 1)):eoff(4 * k)],
                                prev_pt[0:64, 0:504],
                                ring[64:128, 504 * (k - 1) + 126:
                                     504 * (k - 1) + 630])
                    else:
                        # host-merged tail tile: 28,29 evict on DVE (idle
                        # at unit end), 30,31 on Act; single dump at k=31
                        wk = 504 if k < NT - 1 else 252
                        if CtT is None:
                            CtT = cp.tile([128, 4 * 504], bf16, tag="C",
                                          name="CtT")
                        dst = CtT[:, 504 * (k - 28):504 * (k - 28) + wk]
                        if k < 30:
                            nc.vector.tensor_copy(dst, pt[:, 0:wk])
                        else:
                            nc.scalar.copy(dst, pt[:, 0:wk])
                        if k == 28 and u < NU - 1:
                            finish27(prev_pt)
                        if k == NT - 1:
                            nc.sync.dma_start(c_out[b, j, 1], CtT[:])
                    if k == 27:
                        # rows 100..107 complete after merge(26)
                        nc.sync.dma_start(
                            o_out[b, j, :, 85:93, :]
                            .rearrange("m u v -> m (u v)"),
                            E[:, eoff(100):eoff(108)])
                        if u == NU - 1:
                            finish27(pt)
                    prev_pt = pt

                    if k == 9:
                        nc.sync.dma_start(c_out[b, j, 0], CtH[:])
                        # row 15's kh2 part (B s0 of tile 4) for the host
                        nc.sync.dma_start(
                            r15_out[b, j],
                            ring[64:128, 504 * NHOST:504 * NHOST + 126])
                    elif k == 13:
                        # rows 16..51 complete after merge(12)
                        nc.sync.dma_start(
                            o_out[b, j, :, 1:37, :]
                            .rearrange("m u v -> m (u v)"),
                            E[:, 126:4662])
                    elif k == 25:
                        # rows 52..99 complete after merge(24)
                        nc.sync.dma_start(
                            o_out[b, j, :, 37:85, :]
                            .rearrange("m u v -> m (u v)"),
                            E[:, 4662:10710])

                S_cur = S_next

    nc.compile()
    return nc


def kernel(x: np.ndarray, w: np.ndarray) -> np.ndarray:
    from concourse.bass_utils import run_bass_kernel_spmd

    if "nc" not in _cache:
        _cache["nc"] = _build_program()
    nc = _cache["nc"]

    wt, wtl = _build_weights(np.asarray(w, dtype=np.float32))
    xb = np.ascontiguousarray(
        np.asarray(x, dtype=np.float32).astype(ml_dtypes.bfloat16))
    in_maps = [{"x_in": xb[c * BPC:(c + 1) * BPC], "wt_in": wt,
                "wtl_in": wtl}
               for c in range(NCORES)]
    _cache["in_maps"] = in_maps
    res = run_bass_kernel_spmd(nc, in_maps, list(range(NCORES)))

    iorder = [0, 2, 1, 3]
    out = np.empty((B, OUT, G, HO, HO), dtype=np.float32)
    conv = np.empty((OUT, HO, HO), dtype=np.float32)
    for c in range(NCORES):
        oc = np.asarray(res.results[c]["o_out"]).astype(np.float32)
        cc = np.asarray(res.results[c]["c_out"]).astype(np.float32)
        rr = np.asarray(res.results[c]["r15_out"]).astype(np.float32)
        for bi in range(BPC):
            for j in range(4):
                conv[:, 16:112] = oc[bi, j][:, 1:]
                # host merge: A[t][:, s] is row 4K+s, B[t][:, s'] is row
                # 4K+s'-1 (kh2 part); t 0..3 = tiles 0..3, 4..7 = 28..31
                Ah = cc[bi, j, :, 0:64, :].reshape(2, 64, 4, RT, 126)
                Bh = cc[bi, j, :, 64:128, :].reshape(2, 64, 4, RT, 126)
                for r in range(NHOST * RT - 1):
                    kp, sp = (r + 1) // 4, (r + 1) % 4
                    conv[:, r] = Ah[0][:, r // 4, r % 4] + Bh[0][:, kp, sp]
                conv[:, 15] = Ah[0][:, 3, 3] + rr[bi, j]
                for r in range(112, 125):
                    t, s = r // 4 - 28, r % 4
                    kp, sp = (r + 1) // 4 - 28, (r + 1) % 4
                    conv[:, r] = Ah[1][:, t, s] + Bh[1][:, kp, sp]
                conv[:, 125] = Ah[1][:, 3, 1]
                for pos, i in enumerate(iorder):
                    q = (j - i) % 4
                    out[c * BPC + bi, 16 * q:16 * (q + 1), i] = np.rot90(
                        conv[16 * pos:16 * (pos + 1)], k=-i, axes=(-2, -1))
    return out


# revision 23
# speedup vs baseline: 1.1555x; 1.0214x over previous
"""Trainium2 Bass kernel for nn_Conv2dP4P4 (P4->P4 group-equivariant conv).

Math (verified vs reference):
  W2 = w.reshape(64,4,64,3,3).sum(1)                  # tap-sum absorbs the
                                                      # reference's group-sum
  out[b, 16q+m', i] = rot90( conv_valid(x[b,:,j], rot90(W2[16q:16q+16], k=i)),
                             k=-i )   with j = (q+i)%4

Device computes ONLY the conv (all 4 rotation blocks packed in M); the final
rot90(k=-i) of each 16-channel block is pure data movement, applied on the
host after the gather, so the device writes plain m-major conv-layout tiles
at full DMA burst size.

Per core (8 cores, batch-sharded: 2 batches x 4 group elements = 8 units):
  - slab S [128, H*W] bf16: partitions 0:64 = x[b,:,j], 64:128 = x shifted
    one row (+W).
  - 32 psum tiles [128, 504] f32 per unit, RT=4 output rows each (tile 31:
    RT=2 + 3 K=64 singles for the last row's kh=2 taps).  Each tile takes
    THREE K=128, M=128 matmul passes (one per kw):
      M cols   0:64  ("A"): taps (0,kw) via x[a] and (1,kw) via x[a+1]
               -> rows 4k..4k+3
      M cols 64:128  ("B"): tap (2,kw) via x[a+1]  -> rows 4k-1..4k+2
    75% PE utilization vs the 37.5% of a 6-pass M=64 scheme.
  - tiles 4..31: Act evicts the B half psum[64:128] -> bf16 ring[64:128]
    (aligned copy); DVE does ONE fused add per tile:
    E[rows 4k..4k+3] = psum_A[0:64, 0:504] + ring[B slots k.s1..s3,(k+1).s0]
    (the ring keeps consecutive tiles' B halves contiguous, so no separate
    cross-tile "tail" op).  TensorTensor allows mixed PSUM+SBUF inputs at
    different base partitions; both-PSUM or both-SBUF would not.
  - tiles 0..3: HOST-merged.  Act evicts the full [128, 504] psum to bf16,
    DMA'd raw; the host adds the A/B halves.  Putting the host tiles at the
    UNIT START means their PSUM banks free via Act alone, so the next
    unit's tile 8..11 allocations never wait on a DVE merge backlog, and
    DVE gets a 4-tile grace period per unit to drain.
  - E [64, 111*126] bf16 staging (rows 15..125), flushed in three DMAs per
    unit (m-major, >=4.5KB contiguous runs); host up-casts to f32.
"""
import sys
import numpy as np
import ml_dtypes

sys.path.insert(0, "/opt/trn_rl_repo")

B, C, G, H, W = 16, 64, 4, 128, 128
OUT = 64
HO = H - 2  # 126
NCORES = 8
BPC = B // NCORES  # batches per core
RT = 4
NT = 32            # 31 full tiles + 1 two-row tile
NHOST = 4          # tiles 0..3 and 28..31 merged on host; 4..27 device
HW = H * W
ER = HO - NHOST * RT + 1   # rows 15..125 staged in E (111)

_cache = {}


def _build_weights(w: np.ndarray):
    """w: [256, 64, 3, 3] -> (WT [128, 12*128], WTL [64, 12*64]) bf16.

    WT column block (j*3+kw)*128:
      [0:64,   0:64] = LH[j, kw]      (kh0, applied to x[a])
      [64:128, 0:64] = LH[j, 3+kw]    (kh1, applied to x[a+1])
      [64:128,64:128]= LH[j, 6+kw]    (kh2 -> B half, rows a-1)
      [0:64,  64:128]= 0
    WTL column block (j*3+kw)*64 = LH[j, 6+kw] (last-row kh2 singles).
    """
    W2 = w.reshape(OUT, 4, C, 3, 3).sum(axis=1)  # [64, 64, 3, 3] f32
    iorder = [0, 2, 1, 3]
    LH = np.zeros((4, 9, C, 64), dtype=np.float32)  # [j, t, c, mhat]
    for j in range(4):
        for pos, i in enumerate(iorder):
            q = (j - i) % 4
            Ki = np.rot90(W2[16 * q:16 * (q + 1)], k=i, axes=(-2, -1))
            for kh in range(3):
                for kw in range(3):
                    t = kh * 3 + kw
                    LH[j, t, :, pos * 16:(pos + 1) * 16] = Ki[:, :, kh, kw].T
    WT = np.zeros((128, 12 * 128), dtype=np.float32)
    WTL = np.zeros((64, 12 * 64), dtype=np.float32)
    for j in range(4):
        for kw in range(3):
            col = (j * 3 + kw) * 128
            WT[0:64, col:col + 64] = LH[j, kw]
            WT[64:128, col:col + 64] = LH[j, 3 + kw]
            WT[64:128, col + 64:col + 128] = LH[j, 6 + kw]
            WTL[:, (j * 3 + kw) * 64:(j * 3 + kw) * 64 + 64] = LH[j, 6 + kw]
    return (WT.astype(ml_dtypes.bfloat16), WTL.astype(ml_dtypes.bfloat16))


def _build_program():
    import concourse.bass as bass
    import concourse.tile as tile
    from concourse import bacc, mybir

    nc = bacc.Bacc("TRN2", target_bir_lowering=False, debug=False)
    x_in = nc.dram_tensor("x_in", [BPC, C, G, H, W], mybir.dt.bfloat16,
                          kind="ExternalInput").ap()
    wt_in = nc.dram_tensor("wt_in", [128, 12 * 128], mybir.dt.bfloat16,
                           kind="ExternalInput").ap()
    wtl_in = nc.dram_tensor("wtl_in", [64, 12 * 64], mybir.dt.bfloat16,
                            kind="ExternalInput").ap()
    o_out = nc.dram_tensor("o_out", [BPC, G, OUT, 97, HO], mybir.dt.bfloat16,
                           kind="ExternalOutput").ap()
    c_out = nc.dram_tensor("c_out", [BPC, G, 2, 128, 4 * 504],
                           mybir.dt.bfloat16, kind="ExternalOutput").ap()
    r15_out = nc.dram_tensor("r15_out", [BPC, G, 64, HO], mybir.dt.bfloat16,
                             kind="ExternalOutput").ap()

    f32 = mybir.dt.float32
    bf16 = mybir.dt.bfloat16

    with tile.TileContext(nc, trace_sim=False) as tc:
        with tc.tile_pool(name="wtp", bufs=1) as wtp, \
             tc.tile_pool(name="slab", bufs=2) as slabp, \
             tc.tile_pool(name="ep", bufs=2) as ep, \
             tc.tile_pool(name="ring", bufs=1) as ringp, \
             tc.tile_pool(name="cp", bufs=6) as cp, \
             tc.tile_pool(name="psc", bufs=8, space="PSUM") as pscp:

            WT = wtp.tile([128, 12 * 128], bf16)
            # weight DMAs ride the Pool/SWDGE queue: off the HWDGE critical
            # path at startup.  Unit 0 only needs the j=0 columns -- load
            # them first so tile-0 Ldweights unblocks early.
            nc.gpsimd.dma_start(WT[:, 0:384], wt_in[:, 0:384])
            nc.gpsimd.dma_start(WT[:, 384:], wt_in[:, 384:])
            WTL = wtp.tile([64, 12 * 64], bf16)
            nc.gpsimd.dma_start(WTL[:], wtl_in)
            # B-half ring for device-merged tiles, partitions 64:128
            ring = ringp.tile([128, NT * 504], bf16)

            def slab_src(u):
                b, j = u // 4, u % 4
                return x_in[b, :, j].rearrange("c h w -> c (h w)")

            # slab as TWO tiles (cols 0:8200 / 8192:16384, 8-col overlap):
            # tiles 0..15 depend only on the lo tile's writers, so a late
            # hi-half DMA can never stall the next unit's first matmuls
            LOW = 8200
            HIB = HW // 2  # 8192

            def load_slab_piece(Slo, Shi, src, piece):
                if piece == 0:
                    nc.sync.dma_start(Slo[0:64, 0:LOW], src[:, 0:LOW])
                elif piece == 1:
                    nc.sync.dma_start(Slo[64:128, 0:LOW],
                                      src[:, W:W + LOW])
                elif piece == 2:
                    nc.sync.dma_start(Shi[0:64, 0:HW - HIB], src[:, HIB:])
                else:
                    nc.sync.dma_start(Shi[64:128, 0:HW - HIB - W],
                                      src[:, HIB + W:])

            NU = BPC * 4
            S_cur = (slabp.tile([128, LOW + 8], bf16, tag="slo",
                                name="S_lo0"),
                     slabp.tile([128, HW - HIB + 8], bf16, tag="shi",
                                name="S_hi0"))
            # unit 0: a tiny head tile (cols 0:1032) for tiles 0..1 lets the
            # first matmuls start ~3us in, while the full lo/hi load streams
            src0 = slab_src(0)
            # Unit-0 startup is DMA-paced, so load each region ONCE from
            # HBM and build the +W-shifted half with DVE 4x copies (halves
            # the startup DMA and feeds the PE pstate ramp sooner).
            # Head tile covers tiles 0..3; unit 0 never reads S_lo < 2048.
            S_head = wtp.tile([128, 2192], bf16, name="S_head")
            nc.sync.dma_start(S_head[0:64, 0:2184], src0[:, 0:2184])
            nc.vector.tensor_copy(S_head[64:128, 0:2056],
                                  S_head[0:64, W:W + 2056])
            nc.sync.dma_start(S_cur[0][0:64, 2048:5252],
                              src0[:, 2048:5252])
            nc.vector.tensor_copy(S_cur[0][64:128, 2048:5124],
                                  S_cur[0][0:64, 2048 + W:5124 + W])
            nc.sync.dma_start(S_cur[0][0:64, 5252:LOW], src0[:, 5252:LOW])
            nc.vector.tensor_copy(S_cur[0][64:128, 5124:LOW - W],
                                  S_cur[0][0:64, 5124 + W:LOW])
            # shifted cols [8072:8200] = x[8200:8328], beyond the lo span
            nc.sync.dma_start(S_cur[0][64:128, LOW - W:LOW],
                              src0[:, LOW:LOW + W])
            load_slab_piece(S_cur[0], S_cur[1], src0, 2)
            nc.vector.tensor_copy(S_cur[1][64:128, 0:HW - HIB - W],
                                  S_cur[1][0:64, W:HW - HIB])


            for u in range(NU):
                b, j = u // 4, u % 4
                S_lo, S_hi = S_cur
                if u + 1 < NU:
                    S_next = (slabp.tile([128, LOW + 8], bf16, tag="slo",
                                         name="S_lo_next"),
                              slabp.tile([128, HW - HIB + 8], bf16,
                                         tag="shi", name="S_hi_next"))
                else:
                    S_next = None

                E = ep.tile([64, ER * HO], bf16, tag="E")
                prev_pt = None
                CtH = cp.tile([128, 4 * 504], bf16, tag="C", name="CtH")
                CtT = None

                def eoff(r):
                    return (r - 15) * 126

                def finish27(pt27):
                    # row 111 first: its B part (CtT s0) was evicted tiles
                    # ago, so it needn't wait for evict(27); then rows
                    # 108..110 via the ring, then flush rows 108..111
                    nc.vector.tensor_add(
                        E[:, eoff(111):eoff(112)], pt27[0:64, 378:504],
                        CtT[64:128, 0:126])
                    nc.vector.tensor_add(
                        E[:, eoff(108):eoff(111)], pt27[0:64, 0:378],
                        ring[64:128, 504 * 27 + 126:504 * 27 + 504])
                    nc.sync.dma_start(
                        o_out[b, j, :, 93:97, :]
                        .rearrange("m u v -> m (u v)"),
                        E[:, eoff(108):eoff(112)])

                korder = list(range(NT))
                if u == NU - 1:
                    korder = [0, 1, 2, 3, 28, 29, 30, 31] + list(range(4, 28))
                for k in korder:
                    if S_next is not None and k in (0, 2, 5, 8):
                        load_slab_piece(S_next[0], S_next[1],
                                        slab_src(u + 1),
                                        {0: 0, 2: 1, 5: 2, 8: 3}[k])

                    if u == 0 and k < 4:
                        S, off = S_head, 0
                    elif k < 16:
                        S, off = S_lo, 0
                    else:
                        S, off = S_hi, HIB
                    pt = pscp.tile([128, RT * 126], f32, tag="conv")
                    if k < NT - 1:
                        p4 = pt[:].rearrange("m (s x) -> m s x", s=RT)
                        for kw in range(3):
                            base = RT * k * W + kw - off
                            nc.tensor.matmul(
                                p4,
                                WT[:, (j * 3 + kw) * 128:
                                   (j * 3 + kw) * 128 + 128],
                                S[:, base:base + RT * 128]
                                .rearrange("c (s x) -> c s x", s=RT)
                                [:, :, 0:126],
                                start=(kw == 0), stop=(kw == 2),
                                skip_group_check=True)
                    else:
                        # rows 124, 125: two windows + kh2 singles for row 125
                        p2 = pt[:, 0:252].rearrange("m (s x) -> m s x", s=2)
                        for kw in range(3):
                            base = RT * k * W + kw - off
                            nc.tensor.matmul(
                                p2,
                                WT[:, (j * 3 + kw) * 128:
                                   (j * 3 + kw) * 128 + 128],
                                S[:, base:base + 2 * 128]
                                .rearrange("c (s x) -> c s x", s=2)
                                [:, :, 0:126],
                                start=(kw == 0), stop=False,
                                skip_group_check=True)
                        for kw in range(3):
                            base = (H - 1) * W + kw - off
                            nc.tensor.matmul(
                                pt[0:64, 126:252],
                                WTL[:, (j * 3 + kw) * 64:
                                    (j * 3 + kw) * 64 + 64],
                                S[0:64, base:base + 126],
                                start=False, stop=(kw == 2),
                                skip_group_check=True)

                    if k < NHOST:
                        # host-merged head tile: full eviction (single dump
                        # at k=9, after all slab-piece DMAs are queued).
                        # tiles 0,1 evict on DVE (idle at unit start),
                        # 2,3 on Act.
                        dst = CtH[:, 504 * k:504 * (k + 1)]
                        if k < 2:
                            nc.vector.tensor_copy(dst, pt[:])
                        else:
                            nc.scalar.copy(dst, pt[:])
                    elif k < 28:
                        # Act: evict B half into the ring (aligned)
                        nc.scalar.copy(ring[64:128, 504 * k:504 * k + 504],
                                       pt[64:128, 0:504])
                        if k > NHOST:
                            # DVE: fused merge of tile k-1 (rows 4(k-1)..+3)
                            nc.vector.tensor_add(
                                E[:, eoff(4 * (k - 1)):eoff(4 * k)],
                                prev_pt[0:64, 0:504],
                                ring[64:128, 504 * (k - 1) + 126:
                                     504 * (k - 1) + 630])
                    else:
                        # host-merged tail tile: 28,29 evict on DVE (idle
                        # at unit end), 30,31 on Act; single dump at k=31
                        wk = 504 if k < NT - 1 else 252
                        if CtT is None:
                            CtT = cp.tile([128, 4 * 504], bf16, tag="C",
                                          name="CtT")
                        dst = CtT[:, 504 * (k - 28):504 * (k - 28) + wk]
                        if k < 30:
                            nc.vector.tensor_copy(dst, pt[:, 0:wk])
                        else:
                            nc.scalar.copy(dst, pt[:, 0:wk])
                        if k == 28 and u < NU - 1:
                            finish27(prev_pt)
                        if k == NT - 1:
                            nc.sync.dma_start(c_out[b, j, 1], CtT[:])
                    if k == 27:
                        # rows 100..107 complete after merge(26)
                        nc.sync.dma_start(
                            o_out[b, j, :, 85:93, :]
                            .rearrange("m u v -> m (u v)"),
                            E[:, eoff(100):eoff(108)])
                        if u == NU - 1:
                            finish27(pt)
                    prev_pt = pt

                    if k == 9:
                        nc.sync.dma_start(c_out[b, j, 0], CtH[:])
                        # row 15's kh2 part (B s0 of tile 4) for the host
                        nc.sync.dma_start(
                            r15_out[b, j],
                            ring[64:128, 504 * NHOST:504 * NHOST + 126])
                    elif k == 13:
                        # rows 16..51 complete after merge(12)
                        nc.sync.dma_start(
                            o_out[b, j, :, 1:37, :]
                            .rearrange("m u v -> m (u v)"),
                            E[:, 126:4662])
                    elif k == 25:
                        # rows 52..99 complete after merge(24)
                        nc.sync.dma_start(
                            o_out[b, j, :, 37:85, :]
                            .rearrange("m u v -> m (u v)"),
                            E[:, 4662:10710])

                S_cur = S_next

    nc.compile()
    return nc


def kernel(x: np.ndarray, w: np.ndarray) -> np.ndarray:
    from concourse.bass_utils import run_bass_kernel_spmd

    if "nc" not in _cache:
        _cache["nc"] = _build_program()
    nc = _cache["nc"]

    wt, wtl = _build_weights(np.asarray(w, dtype=np.float32))
    xb = np.ascontiguousarray(
        np.asarray(x, dtype=np.float32).astype(ml_dtypes.bfloat16))
    in_maps = [{"x_in": xb[c * BPC:(c + 1) * BPC], "wt_in": wt,
                "wtl_in": wtl}
               for c in range(NCORES)]
    _cache["in_maps"] = in_maps
    res = run_bass_kernel_spmd(nc, in_maps, list(range(NCORES)))

    iorder = [0, 2, 1, 3]
    out = np.empty((B, OUT, G, HO, HO), dtype=np.float32)
    conv = np.empty((OUT, HO, HO), dtype=np.float32)
    for c in range(NCORES):
        oc = np.asarray(res.results[c]["o_out"]).astype(np.float32)
        cc = np.asarray(res.results[c]["c_out"]).astype(np.float32)
        rr = np.asarray(res.results[c]["r15_out"]).astype(np.float32)
        for bi in range(BPC):
            for j in range(4):
                conv[:, 16:112] = oc[bi, j][:, 1:]
                # host merge: A[t][:, s] is row 4K+s, B[t][:, s'] is row
                # 4K+s'-1 (kh2 part); t 0..3 = tiles 0..3, 4..7 = 28..31
                Ah = cc[bi, j, :, 0:64, :].reshape(2, 64, 4, RT, 126)
                Bh = cc[bi, j, :, 64:128, :].reshape(2, 64, 4, RT, 126)
                for r in range(NHOST * RT - 1):
                    kp, sp = (r + 1) // 4, (r + 1) % 4
                    conv[:, r] = Ah[0][:, r // 4, r % 4] + Bh[0][:, kp, sp]
                conv[:, 15] = Ah[0][:, 3, 3] + rr[bi, j]
                for r in range(112, 125):
                    t, s = r // 4 - 28, r % 4
                    kp, sp = (r + 1) // 4 - 28, (r + 1) % 4
                    conv[:, r] = Ah[1][:, t, s] + Bh[1][:, kp, sp]
                conv[:, 125] = Ah[1][:, 3, 1]
                for pos, i in enumerate(iorder):
                    q = (j - i) % 4
                    out[c * BPC + bi, 16 * q:16 * (q + 1), i] = np.rot90(
                        conv[16 * pos:16 * (pos + 1)], k=-i, axes=(-2, -1))
    return out
